# revision 1
# baseline (speedup 1.0000x reference)
"""Trainium2 Bass kernel for nn_GCNNLayer_56796647522692 (GCN message-passing layer).

Math (per flattened token row j of M = BNK*L = 25600, D = O = 1024, R = 50):
    idx      = adj_arc_in[:,0]*L + adj_arc_in[:,1]          (gather source rows)
    in_      = rep_[idx] @ W_in + b_in[lab]                 (gather commutes with matmul)
    in_gate  = rep_[idx] @ W_gate_in + b_gate_in[lab]
    same_    = rep_ @ W_self
    same_g   = rep_ @ W_gate_self
    w_in     = adj_mask_in^2  * sigmoid(in_gate)
    w_self   = adj_mask_loop^2 * sigmoid(same_g)
    out      = relu(in_*w_in + same_*w_self) * mask

Sharding: data-parallel over rows, 3200 rows/core on 8 cores. The host performs the
row gather (rep_[idx]), the lhsT-layout transposes, the label one-hot, and the mask
packing during input sharding; each core then runs a dense fused matmul kernel.

Device layout (token-major outputs, out partitions = tokens):
    lhsT = X^T 128x128 chunks (stationary), rhs = W 128x512 chunks (moving),
    PSUM accumulates over the 8 k-tiles (fp16 inputs, fp32 accumulate; ~5e-4 rel
    err).  Gates ride the same lhsT as N=2 matmuls; b_gate_in[lab] is applied as
    the sigmoid's per-partition bias; b_in[lab] rows are host-gathered and added
    on VectorE.
    Combine on ACT/DVE: sigmoid -> per-partition scales -> relu*mask -> DMA out.
    Steady state is TensorE-saturated: 800 N=512 matmuls/core = 171us stream time,
    ~200us modeled end-to-end per core.
"""

import numpy as np
import ml_dtypes

import concourse.bass as bass
import concourse.tile as tile
from concourse import bacc, mybir
from concourse.bass_utils import run_bass_kernel_spmd

# ---- problem dims (hardcoded per contract) ----
BNK, L, D, O, R = 200, 128, 1024, 1024, 50
M = BNK * L              # 25600
NCORES = 8
MC = M // NCORES         # 3200 rows per core
P = 128
MT = MC // P             # 25 m-tiles per core
KT = D // P              # 8 k-tiles
NFREE = 512
NT = O // NFREE          # 2 n-chunks

# matmul input dtype: "bf16"/"f16" (1 cyc/row), "f32r" (1 cyc/row at N>=256,
# tf32-class precision), "f32" (4 cyc/row, full fp32)
import os
MM_MODE = os.environ.get("GCN_MM_MODE", "f16")
# bench-only: repeat the whole compute loop R times inside the NEFF so kernel
# time dominates the per-exec RPC overhead; slope between two R values gives HW time
REPEAT = int(os.environ.get("GCN_REPEAT", "1"))
# b_in[lab] add: "pe" = one-hot matmul on TensorE, "dve" = host-gathered rows
# added on VectorE (saves ~10us of PE time, costs extra DMA + DVE)
BIAS = os.environ.get("GCN_BIAS", "dve")
# timing probe only (wrong math): skip gate matmuls to measure their PE cost
NOGATE = os.environ.get("GCN_NOGATE", "0") == "1"
# PSUM slots: big pool bufs / gate pool bufs (8 banks total)
PSUM_BIG = int(os.environ.get("GCN_PSUM_BIG", "6"))
PSUM_G = int(os.environ.get("GCN_PSUM_G", "2"))
# per-m-tile emission order: "batch" = all I then all S, combine at end;
# "nphase" = per n-chunk {I_n, S_n, combine_n} so psum banks free earlier
ORDER = os.environ.get("GCN_ORDER", "nphase")

_DT = {
    "bf16": (mybir.dt.bfloat16, ml_dtypes.bfloat16),
    "f16": (mybir.dt.float16, np.float16),
    "f32r": (mybir.dt.float32r, np.float32),
    "f32": (mybir.dt.float32, np.float32),
}
MM_DT, MM_NP = _DT[MM_MODE]
F32 = mybir.dt.float32
AF = mybir.ActivationFunctionType


def build_bass():
    nc = bacc.Bacc("TRN2", target_bir_lowering=False, debug=False, num_devices=NCORES)

    xt = nc.dram_tensor("xt", (MT, P, 2, KT, P), MM_DT, kind="ExternalInput").ap()
    w = nc.dram_tensor("w", (2, D, O), MM_DT, kind="ExternalInput").ap()
    wg = nc.dram_tensor("wg", (D, 2), MM_DT, kind="ExternalInput").ap()
    if BIAS == "pe":
        bau = nc.dram_tensor("bau", (R, O), MM_DT, kind="ExternalInput").ap()
        oht = nc.dram_tensor("oht", (R, MC), MM_DT, kind="ExternalInput").ap()
    msk = nc.dram_tensor("msk", (P, MT, 6), F32, kind="ExternalInput").ap()
    if BIAS == "dve":
        brow = nc.dram_tensor("brow", (MT, P, O), MM_DT, kind="ExternalInput").ap()
    out = nc.dram_tensor("out", (MC, O), F32, kind="ExternalOutput").ap()

    with tile.TileContext(nc) as tc:
        with (
            tc.tile_pool(name="const", bufs=1) as const,
            tc.tile_pool(name="xtp", bufs=4) as xtp,
            tc.tile_pool(name="colp", bufs=4) as colp,
            tc.tile_pool(name="tmp", bufs=4) as tmpp,
            tc.tile_pool(name="outp", bufs=4) as outp,
            tc.tile_pool(name="psum", bufs=PSUM_BIG, space="PSUM") as psum,
            tc.tile_pool(name="psg", bufs=PSUM_G, space="PSUM") as psg,
        ):
            # first m-tile's inputs and the small constants before the 4MB weight
            # preload so the first matmuls are not queued behind it
            xt0 = xtp.tile([P, 2, KT, P], MM_DT, tag="xt_t", name="xt0")
            nc.sync.dma_start(xt0[:], xt[0])
            br0 = None
            if BIAS == "dve":
                br0 = xtp.tile([P, O], MM_DT, tag="brow", name="br0")
                nc.sync.dma_start(br0[:], brow[0])
            wg_sb = const.tile([P, KT, 2], MM_DT)
            nc.sync.dma_start(wg_sb[:], wg.rearrange("(k p) g -> p k g", p=P))
            if BIAS == "pe":
                bau_sb = const.tile([R, O], MM_DT)
                nc.sync.dma_start(bau_sb[:], bau)
                oht_sb = const.tile([R, MC], MM_DT)
                nc.sync.dma_start(oht_sb[:], oht)
            msk_sb = const.tile([P, MT, 6], F32)
            nc.sync.dma_start(msk_sb[:], msk)

            # ---- weight preload ----
            # per-(s,k) weight tiles so the first matmuls only wait on the first chunk
            w_t = [[const.tile([P, O], MM_DT, name=f"w_{s}_{k}") for k in range(KT)]
                   for s in range(2)]
            for k in range(KT):
                for s in range(2):
                    nc.sync.dma_start(w_t[s][k][:], w[s, k * P:(k + 1) * P, :])

            first = True
            for m in [mm for _ in range(REPEAT) for mm in range(MT)]:
                if first and m == 0:
                    xt_t, br_t, first = xt0, br0, False
                else:
                    xt_t = xtp.tile([P, 2, KT, P], MM_DT, tag="xt_t", name="xt_t")
                    nc.sync.dma_start(xt_t[:], xt[m])
                    if BIAS == "dve":
                        br_t = xtp.tile([P, O], MM_DT, tag="brow", name="br_t")
                        nc.sync.dma_start(br_t[:], brow[m])

                # gate psum: cols 0:2 = Xin @ [wg_in, wg_self], cols 2:4 = Xself @ same.
                # Only col 0 (g_in) and col 3 (g_self) are used; N=2 because f32r
                # matmuls reject a single-element free dim.
                g_ps = psg.tile([P, 4], F32)
                oh_m = oht_sb[:, m * P:(m + 1) * P] if BIAS == "pe" else None
                wcol = colp.tile([P, 4], F32)

                def mm_block(n, s, with_gates, m=m, xt_t=xt_t, g_ps=g_ps, oh_m=oh_m):
                    """8 k-tile matmuls of source s into a fresh psum tile for
                    n-chunk n; optionally ride the gate matmuls on the same lhsT."""
                    ps = psum.tile([P, NFREE], F32, tag="big", name=f"ps{s}{n}")
                    gsl = slice(0, 2) if s == 0 else slice(2, 4)
                    for k in range(KT):
                        lhsT = xt_t[:, s, k]
                        last = k == KT - 1
                        nc.tensor.matmul(
                            ps[:], lhsT, w_t[s][k][:, n * NFREE:(n + 1) * NFREE],
                            start=(k == 0),
                            stop=(last and (s == 1 or BIAS == "dve")))
                        if with_gates and not NOGATE:
                            nc.tensor.matmul(
                                g_ps[:, gsl], lhsT, wg_sb[:, k, 0:2],
                                start=(k == 0), stop=last)
                    if BIAS == "pe" and s == 0:
                        nc.tensor.matmul(
                            ps[:], oh_m, bau_sb[:, n * NFREE:(n + 1) * NFREE],
                            start=False, stop=True)
                    return ps

                def finish_gates(m=m, g_ps=g_ps, wcol=wcol):
                    # gate weights: w = mask_soft^2 * sigmoid(gate + gate_bias);
                    # cols 0, 3 valid.  b_gate_in[lab] rides msk col 5 and is applied
                    # as the sigmoid's per-partition bias (in-cols only).
                    if NOGATE:
                        nc.vector.tensor_copy(wcol[:], msk_sb[:, m, 0:4])
                    else:
                        nc.scalar.activation(wcol[:, 0:2], g_ps[:, 0:2], AF.Sigmoid,
                                             bias=msk_sb[:, m, 5:6])
                        nc.scalar.activation(wcol[:, 2:4], g_ps[:, 2:4], AF.Sigmoid)
                        nc.vector.tensor_tensor(wcol[:], wcol[:], msk_sb[:, m, 0:4],
                                                mybir.AluOpType.mult)

                def combine(n, ips, sps, m=m, wcol=wcol):
                    # out = relu((I+b)*w_in + S*w_self) * mask
                    t1 = tmpp.tile([P, NFREE], F32, tag="t1", name="t1")
                    t2 = tmpp.tile([P, NFREE], F32, tag="t2", name="t2")
                    if BIAS == "dve":
                        nc.vector.tensor_tensor(
                            t1[:], ips[:], br_t[:, n * NFREE:(n + 1) * NFREE],
                            mybir.AluOpType.add)
                        nc.scalar.mul(t1[:], t1[:], wcol[:, 0:1])
                    else:
                        nc.scalar.mul(t1[:], ips[:], wcol[:, 0:1])
                    nc.vector.tensor_scalar_mul(t2[:], sps[:], wcol[:, 3:4])
                    nc.vector.tensor_add(t1[:], t1[:], t2[:])
                    o_t = outp.tile([P, NFREE], F32, tag="ot", name="o_t")
                    nc.scalar.activation(o_t[:], t1[:], AF.Relu,
                                         scale=msk_sb[:, m, 4:5])
                    nc.sync.dma_start(
                        out[m * P:(m + 1) * P, n * NFREE:(n + 1) * NFREE], o_t[:])

                if ORDER == "batch":
                    i_ps = [mm_block(n, 0, with_gates=(n == 0)) for n in range(NT)]
                    s_ps = [mm_block(n, 1, with_gates=(n == 0)) for n in range(NT)]
                    finish_gates()
                    for n in range(NT):
                        combine(n, i_ps[n], s_ps[n])
                else:  # nphase: free each n-chunk's psum banks before the next
                    i0 = mm_block(0, 0, with_gates=True)
                    s0 = mm_block(0, 1, with_gates=True)
                    finish_gates()
                    combine(0, i0, s0)
                    i1 = mm_block(1, 0, with_gates=False)
                    s1 = mm_block(1, 1, with_gates=False)
                    combine(1, i1, s1)

    nc.compile()
    return nc


_NC = None


def _get_nc():
    global _NC
    if _NC is None:
        _NC = build_bass()
    return _NC


def make_in_maps(rep, adj_arc_in, adj_lab_in, adj_mask_in, adj_mask_loop, mask,
                 W_in, b_in, W_gate_in, b_gate_in, W_self, W_gate_self):
    rep_ = np.ascontiguousarray(np.asarray(rep, dtype=np.float32)).reshape(M, D)
    arc = np.asarray(adj_arc_in)
    lab = np.asarray(adj_lab_in)
    idx = arc[:, 0].astype(np.int64) * L + arc[:, 1].astype(np.int64)
    gath = rep_[idx]  # (M, D)

    w_both = np.stack([np.asarray(W_in), np.asarray(W_self)]).astype(MM_NP)
    wg2 = np.concatenate([np.asarray(W_gate_in), np.asarray(W_gate_self)],
                         axis=1).astype(MM_NP)
    bg = np.asarray(b_gate_in, dtype=np.float32)[:, 0]

    m2i = (np.asarray(adj_mask_in)[:, 0].astype(np.float32)) ** 2
    m2l = (np.asarray(adj_mask_loop)[:, 0].astype(np.float32)) ** 2
    mk = np.asarray(mask, dtype=np.float32).reshape(M)

    in_maps = []
    for c in range(NCORES):
        rows = slice(c * MC, (c + 1) * MC)
        xb = np.stack([gath[rows], rep_[rows]])          # (2, MC, D) [s, j, d]
        v = xb.reshape(2, MT, P, KT, P)                  # [s, m, c, k, p]
        xt_c = np.ascontiguousarray(v.transpose(1, 4, 0, 3, 2)).astype(MM_NP)
        zc = np.zeros((P, MT), np.float32)
        msk_c = np.ascontiguousarray(np.stack(
            [m2i[rows].reshape(MT, P).T, zc, zc,
             m2l[rows].reshape(MT, P).T,
             mk[rows].reshape(MT, P).T,
             bg[lab[rows]].reshape(MT, P).T], axis=2)).astype(np.float32)
        im = {"xt": xt_c, "w": w_both, "wg": wg2, "msk": msk_c}
        if BIAS == "pe":
            im["bau"] = np.asarray(b_in, dtype=np.float32).astype(MM_NP)
            im["oht"] = (np.asarray(lab[rows])[None, :] ==
                         np.arange(R)[:, None]).astype(MM_NP)
        if BIAS == "dve":
            im["brow"] = np.asarray(b_in, dtype=np.float32)[
                lab[rows]].astype(MM_NP).reshape(MT, P, O)
        in_maps.append(im)
    return in_maps


def kernel(**inputs):
    import time
    nc = _get_nc()
    in_maps = make_in_maps(**inputs)
    last = None
    for attempt in range(3):
        try:
            res = run_bass_kernel_spmd(nc, in_maps, core_ids=list(range(NCORES)))
            out = np.concatenate(
                [np.asarray(res.results[c]["out"]) for c in range(NCORES)], axis=0)
            return out.reshape(BNK, L, O)
        except Exception as e:  # transient device/tunnel errors: back off and retry
            last = e
            time.sleep(20 * (attempt + 1))
    raise last



# revision 6
# speedup vs baseline: 2.2959x; 2.2959x over previous
"""Trainium2 Bass kernel for nn_GCNNLayer_56796647522692 (GCN message-passing layer).

Math (per flattened token row j of M = BNK*L = 25600, D = O = 1024, R = 50):
    idx      = adj_arc_in[:,0]*L + adj_arc_in[:,1]          (gather source rows)
    in_      = rep_[idx] @ W_in + b_in[lab]
    in_gate  = rep_[idx] @ W_gate_in + b_gate_in[lab]
    same_    = rep_ @ W_self
    same_g   = rep_ @ W_gate_self
    w_in     = adj_mask_in^2  * sigmoid(in_gate)
    w_self   = adj_mask_loop^2 * sigmoid(same_g)
    out      = relu(in_*w_in + same_*w_self) * mask

Strategy: the gates/sigmoids/masks are O(M*D) host work, so they are folded
into the inputs on the host: each token's gathered row is pre-scaled by
w_in*mask and its self row by w_self*mask (relu(x*m) = relu(x)*m for m>=0),
making the device computation a single fused accumulation
    out_row = relu([x_in*w_in | x_self*w_self] @ [W_in; W_self])
over a 2048-wide contraction into one PSUM bank, followed by one ACT relu.
Tokens are reordered by class: dead tokens (w_in=w_self=0, ~10%) are skipped
entirely; self-only tokens (w_in=0, ~9%) contract only their 1024 self
features.  The first P8 feature-pair k-tiles of each class-AB tile run as
fp8e4 DoubleRow matmuls (2 contraction rows/cycle); the rest ride f16.
P8=1 measures 1.4e-2 rel err on the reference distribution (f16-only 3e-4,
fp8-only 3.9e-2 vs the 2e-2 gate).

Sharding: data-parallel over tokens, 3200 rows/core on 8 cores; weights
replicated. Output rows are DMA'd f16 and re-permuted/zero-filled on host.
"""

import os
import numpy as np
import ml_dtypes

import concourse.bass as bass
import concourse.tile as tile
from concourse import bacc, mybir
from concourse.bass_utils import run_bass_kernel_spmd

# ---- problem dims (hardcoded per contract) ----
BNK, L, D, O, R = 200, 128, 1024, 1024, 50
M = BNK * L              # 25600
NCORES = 8
MC = M // NCORES         # 3200 rows per core
P = 128
KT = D // P              # 8 k-tiles per source
NFREE = 512
NT = O // NFREE          # 2 n-chunks

# number of feature-pair k-tiles (2*128 contraction rows each) per AB tile
# that run as fp8e4 DoubleRow instead of two f16 matmuls (0..8)
P8 = int(os.environ.get("GCN_P8", "1"))
# bench-only: repeat the whole compute loop R times inside the NEFF so kernel
# time dominates per-exec RPC overhead; slope between two R values = HW time
REPEAT = int(os.environ.get("GCN_REPEAT", "1"))

F32 = mybir.dt.float32
F16 = mybir.dt.float16
F8 = mybir.dt.float8e4
AF = mybir.ActivationFunctionType
DR = mybir.MatmulPerfMode.DoubleRow
NP_F8 = ml_dtypes.float8_e4m3


def build_bass(ta, ts, ti, with_bias, p8):
    """ta/ts/ti = AB / self-only / in-only tile counts (128 tokens each)."""
    kf = KT - p8                 # f16 k-tiles per source half in AB tiles
    nc = bacc.Bacc("TRN2", target_bir_lowering=False, debug=False,
                   num_devices=NCORES)

    # AB tiles: fp8 pair part [k, i, ko, tok] and f16 part [k, j, tok] where
    # j < kf is W_in tile p8+j, j >= kf is W_self tile p8+(j-kf)
    xa8 = xa16 = xs = xi = None
    if ta and p8:
        xa8 = nc.dram_tensor("xa8", (ta, P, p8, 2, P), F8, kind="ExternalInput").ap()
    if ta:
        xa16 = nc.dram_tensor("xa16", (ta, P, 2 * kf, P), F16, kind="ExternalInput").ap()
    if ts:
        xs = nc.dram_tensor("xs", (ts, P, KT, P), F16, kind="ExternalInput").ap()
    if ti:
        xi = nc.dram_tensor("xi", (ti, P, KT, P), F16, kind="ExternalInput").ap()
    # weights: fp8 pairs [i, k, ko, o]; f16 W_in tiles p8..8; full f16 W_self
    w8 = nc.dram_tensor("w8", (max(p8, 1), P, 2, O), F8, kind="ExternalInput").ap()
    wi = nc.dram_tensor("wi", (KT, P, O), F16, kind="ExternalInput").ap()
    ws = nc.dram_tensor("ws", (KT, P, O), F16, kind="ExternalInput").ap()
    brow = None
    if with_bias:
        brow = nc.dram_tensor("brow", (ta + ti, P, O), F16, kind="ExternalInput").ap()
    oab = nc.dram_tensor("oab", (max(ta, 1) * P, O), F16, kind="ExternalOutput").ap()
    osf = nc.dram_tensor("osf", (max(ts, 1) * P, O), F16, kind="ExternalOutput").ap()
    oin = nc.dram_tensor("oin", (max(ti, 1) * P, O), F16, kind="ExternalOutput").ap()

    with tile.TileContext(nc) as tc:
        with (
            tc.tile_pool(name="const", bufs=1) as const,
            tc.tile_pool(name="xtp", bufs=4) as xtp,
            tc.tile_pool(name="outp", bufs=4) as outp,
            tc.tile_pool(name="psum", bufs=6, space="PSUM") as psum,
        ):
            # first AB tile's inputs before the weight preload so the first
            # matmuls are not queued behind 5MB of weight DMA
            x80 = x160 = None
            if ta:
                if p8:
                    x80 = xtp.tile([P, p8, 2, P], F8, tag="x8", name="x80")
                    nc.sync.dma_start(x80[:], xa8[0])
                x160 = xtp.tile([P, 2 * kf, P], F16, tag="x16", name="x160")
                nc.sync.dma_start(x160[:], xa16[0])

            w8_sb = const.tile([P, max(p8, 1), 2, O], F8)
            nc.sync.dma_start(w8_sb[:], w8.rearrange("i k t o -> k i t o"))
            wi_sb = [const.tile([P, O], F16, name=f"wi{k}") for k in range(KT)]
            ws_sb = [const.tile([P, O], F16, name=f"ws{k}") for k in range(KT)]
            for k in range(KT):
                nc.sync.dma_start(wi_sb[k][:], wi[k])
                nc.sync.dma_start(ws_sb[k][:], ws[k])

            def emit(x8_t, x16_t, br_t, out_dram, t, nf16, wlist, p8_here):
                """One 128-token tile: accumulate + relu + store both n-chunks."""
                for n in range(NT):
                    nsl = slice(n * NFREE, (n + 1) * NFREE)
                    ps = psum.tile([P, NFREE], F32, tag="ps", name="ps")
                    nmm = p8_here + nf16
                    mi = 0
                    for i in range(p8_here):
                        nc.tensor.matmul(ps[:], x8_t[:, i], w8_sb[:, i, :, nsl],
                                         start=(mi == 0), stop=(mi == nmm - 1),
                                         perf_mode=DR)
                        mi += 1
                    for j in range(nf16):
                        nc.tensor.matmul(ps[:], x16_t[:, j], wlist[j][:, nsl],
                                         start=(mi == 0), stop=(mi == nmm - 1))
                        mi += 1
                    o_t = outp.tile([P, NFREE], F16, tag="ot", name="ot")
                    if br_t is not None:
                        tv = outp.tile([P, NFREE], F32, tag="tv", name="tv")
                        nc.vector.tensor_tensor(tv[:], ps[:], br_t[:, nsl],
                                                mybir.AluOpType.add)
                        nc.scalar.activation(o_t[:], tv[:], AF.Relu)
                    else:
                        nc.scalar.activation(o_t[:], ps[:], AF.Relu)
                    nc.sync.dma_start(out_dram[t * P:(t + 1) * P, nsl], o_t[:])

            first = True
            for _ in range(REPEAT):
                for t in range(ta):
                    if first:
                        x8_t, x16_t, first = x80, x160, False
                    else:
                        x8_t = None
                        if p8:
                            x8_t = xtp.tile([P, p8, 2, P], F8, tag="x8", name="x8")
                            nc.sync.dma_start(x8_t[:], xa8[t])
                        x16_t = xtp.tile([P, 2 * kf, P], F16, tag="x16", name="x16")
                        nc.sync.dma_start(x16_t[:], xa16[t])
                    br_t = None
                    if with_bias:
                        br_t = xtp.tile([P, O], F16, tag="br", name="br")
                        nc.sync.dma_start(br_t[:], brow[t])
                    wlist = wi_sb[p8:] + ws_sb[p8:]
                    emit(x8_t, x16_t, br_t, oab, t, 2 * kf, wlist, p8)
                for t in range(ts):
                    xs_t = xtp.tile([P, KT, P], F16, tag="x16", name="xs_t")
                    nc.sync.dma_start(xs_t[:], xs[t])
                    emit(None, xs_t, None, osf, t, KT, ws_sb, 0)
                for t in range(ti):
                    xi_t = xtp.tile([P, KT, P], F16, tag="x16", name="xi_t")
                    nc.sync.dma_start(xi_t[:], xi[t])
                    br_t = None
                    if with_bias:
                        br_t = xtp.tile([P, O], F16, tag="br", name="br2")
                        nc.sync.dma_start(br_t[:], brow[ta + t])
                    emit(None, xi_t, br_t, oin, t, KT, wi_sb, 0)

    nc.compile()
    return nc


_NC_CACHE = {}


def _get_nc(key):
    if key not in _NC_CACHE:
        _NC_CACHE[key] = build_bass(*key)
    return _NC_CACHE[key]


def make_in_maps(rep, adj_arc_in, adj_lab_in, adj_mask_in, adj_mask_loop, mask,
                 W_in, b_in, W_gate_in, b_gate_in, W_self, W_gate_self):
    rep_ = np.ascontiguousarray(np.asarray(rep, dtype=np.float32)).reshape(M, D)
    arc = np.asarray(adj_arc_in)
    lab = np.asarray(adj_lab_in)
    idx = arc[:, 0].astype(np.int64) * L + arc[:, 1].astype(np.int64)
    gath = rep_[idx]                                  # (M, D)

    # host-side gates -> per-token combine weights (exact f32 math)
    g_in = gath @ np.asarray(W_gate_in, np.float32) + \
        np.asarray(b_gate_in, np.float32)[lab]
    g_self = rep_ @ np.asarray(W_gate_self, np.float32)
    sig = lambda x: 1.0 / (1.0 + np.exp(-x))
    mk = np.asarray(mask, np.float32).reshape(M)
    w_in = (np.asarray(adj_mask_in, np.float32)[:, 0] ** 2) * sig(g_in[:, 0]) * mk
    w_self = (np.asarray(adj_mask_loop, np.float32)[:, 0] ** 2) * sig(g_self[:, 0]) * mk

    b_np = np.asarray(b_in, np.float32)
    with_bias = bool(np.any(b_np))

    win = np.asarray(W_in, np.float32)
    wself = np.asarray(W_self, np.float32)
    # fp8 weight pairs [i, k, ko, o]: ko=0 -> W_in tile i, ko=1 -> W_self tile i
    w8 = np.stack([win.reshape(KT, P, O)[:P8], wself.reshape(KT, P, O)[:P8]],
                  axis=2).astype(NP_F8) if P8 else \
        np.zeros((1, P, 2, O), NP_F8)
    wi16 = win.reshape(KT, P, O).astype(np.float16)
    ws16 = wself.reshape(KT, P, O).astype(np.float16)

    xin_s = gath * w_in[:, None]
    xsf_s = rep_ * w_self[:, None]

    in_maps, metas = [], []
    for c in range(NCORES):
        rows = np.arange(c * MC, (c + 1) * MC)
        ain = w_in[rows] != 0
        asf = w_self[rows] != 0
        r_ab = rows[ain & asf]
        r_sf = rows[~ain & asf]
        r_in = rows[ain & ~asf]
        ta = -(-len(r_ab) // P) if len(r_ab) else 0
        ts = -(-len(r_sf) // P) if len(r_sf) else 0
        ti = -(-len(r_in) // P) if len(r_in) else 0

        im = {"w8": w8, "wi": wi16, "ws": ws16}
        if ta:
            xcat = np.concatenate([xin_s[r_ab], xsf_s[r_ab]], axis=1)
            pad = ta * P - len(r_ab)
            if pad:
                xcat = np.concatenate([xcat, np.zeros((pad, 2 * D), np.float32)])
            v = xcat.reshape(ta, P, 2 * KT, P).transpose(0, 3, 2, 1)
            if P8:
                pairs = np.stack([v[:, :, 0:P8], v[:, :, KT:KT + P8]], axis=3)
                im["xa8"] = np.ascontiguousarray(pairs).astype(NP_F8)
            f16_k = list(range(P8, KT)) + list(range(KT + P8, 2 * KT))
            im["xa16"] = np.ascontiguousarray(v[:, :, f16_k]).astype(np.float16)
        if ts:
            xc = xsf_s[r_sf]
            pad = ts * P - len(r_sf)
            if pad:
                xc = np.concatenate([xc, np.zeros((pad, D), np.float32)])
            v = xc.reshape(ts, P, KT, P).transpose(0, 3, 2, 1)
            im["xs"] = np.ascontiguousarray(v).astype(np.float16)
        if ti:
            xc = xin_s[r_in]
            pad = ti * P - len(r_in)
            if pad:
                xc = np.concatenate([xc, np.zeros((pad, D), np.float32)])
            v = xc.reshape(ti, P, KT, P).transpose(0, 3, 2, 1)
            im["xi"] = np.ascontiguousarray(v).astype(np.float16)
        if with_bias:
            br = np.zeros(((ta + ti) * P, O), np.float32)
            if len(r_ab):
                br[:len(r_ab)] = b_np[lab[r_ab]] * w_in[r_ab][:, None]
            if len(r_in):
                br[ta * P:ta * P + len(r_in)] = \
                    b_np[lab[r_in]] * w_in[r_in][:, None]
            im["brow"] = br.reshape(ta + ti, P, O).astype(np.float16)
        in_maps.append(im)
        metas.append((ta, ts, ti, r_ab, r_sf, r_in))
    return in_maps, metas, with_bias


def prepare(inputs):
    """make_in_maps + compile + pad all cores to shared tile counts."""
    in_maps, metas, with_bias = make_in_maps(**inputs)
    # tile counts are data-dependent; compile one program per shape tuple
    # (all cores share one SPMD program, so use the max counts and pad)
    ta = max(m[0] for m in metas)
    ts = max(m[1] for m in metas)
    ti = max(m[2] for m in metas)
    key = (ta, ts, ti, with_bias, P8)
    nc = _get_nc(key)

    # pad each core's arrays up to the shared (ta, ts, ti)
    for im, (cta, cts, cti, *_rest) in zip(in_maps, metas):
        if ta:
            if P8:
                a = im.get("xa8", np.zeros((0, P, P8, 2, P), NP_F8))
                if len(a) < ta:
                    im["xa8"] = np.concatenate(
                        [a, np.zeros((ta - len(a), P, P8, 2, P), NP_F8)])
            a = im.get("xa16", np.zeros((0, P, 2 * (KT - P8), P), np.float16))
            if len(a) < ta:
                im["xa16"] = np.concatenate(
                    [a, np.zeros((ta - len(a), P, 2 * (KT - P8), P), np.float16)])
        if ts:
            a = im.get("xs", np.zeros((0, P, KT, P), np.float16))
            if len(a) < ts:
                im["xs"] = np.concatenate(
                    [a, np.zeros((ts - len(a), P, KT, P), np.float16)])
        if ti:
            a = im.get("xi", np.zeros((0, P, KT, P), np.float16))
            if len(a) < ti:
                im["xi"] = np.concatenate(
                    [a, np.zeros((ti - len(a), P, KT, P), np.float16)])
        if with_bias:
            a = im.get("brow", np.zeros((0, P, O), np.float16))
            if len(a) < ta + ti:
                im["brow"] = np.concatenate(
                    [a, np.zeros((ta + ti - len(a), P, O), np.float16)])
    return nc, in_maps, metas


def kernel(**inputs):
    import time
    nc, in_maps, metas = prepare(inputs)

    last = None
    for attempt in range(3):
        try:
            res = run_bass_kernel_spmd(nc, in_maps, core_ids=list(range(NCORES)))
            break
        except Exception as e:  # transient device/tunnel errors: back off, retry
            last = e
            time.sleep(20 * (attempt + 1))
    else:
        raise last

    out = np.zeros((M, O), np.float32)
    for c in range(NCORES):
        r = res.results[c]
        _, _, _, r_ab, r_sf, r_in = metas[c]
        if len(r_ab):
            out[r_ab] = np.asarray(r["oab"][:len(r_ab)], np.float32)
        if len(r_sf):
            out[r_sf] = np.asarray(r["osf"][:len(r_sf)], np.float32)
        if len(r_in):
            out[r_in] = np.asarray(r["oin"][:len(r_in)], np.float32)
    return out.reshape(BNK, L, O)
